# revision 35
# baseline (speedup 1.0000x reference)
"""RWKV-style Block kernel for 8 Trainium2 NeuronCores (batch-parallel SPMD).

v4: software-pipelined main loop with DEFERRED sh/wo matmuls (one iteration
late, so the PE never waits on the current chunk's WKV recurrence), host-side
LN1 statistics (device does only the affine normalize), the short-cut mean
term folded into the matmul as a rank-1 update, PE transposes, batched Act
tables, staged SBUF pools so phase-3 weights load during the last chunk tail.

Iteration j emission (chunks of 512 tokens):
  A1(j+1):  x bf16 -> hn = (x-mu)*rstd (Act, host stats) -> PE-transpose
  MIX(j+1): time-mixes xk/xv/xr
  B1a(j):   r matmuls -> sigmoid batch
  B1c(j):   k/v matmuls -> Exp batch -> WKV scan -> rw
  A2a(j-1): short matmuls (+rank-1 mu/std*srow) -> o1=ps*std; wo matmuls,
            o1 += att; spill o1
  A2b(j-1): ln2 -> gn -> PE-transpose -> ffn mixes -> spill gkT/grT
Phase 3 per chunk: fwr matmul -> sigmoid sr2 -> PE-transpose [t,c];
  kk2 = relu(fwk@gk+b)^2 -> kv matmuls -> out = o1 + sr2*kv
"""
import numpy as np
import ml_dtypes

import concourse.bass as bass
import concourse.bacc as bacc
import concourse.mybir as mybir
import concourse.tile as tile
from concourse.bass_utils import run_bass_kernel_spmd

F32 = mybir.dt.float32
BF16 = mybir.dt.bfloat16
AL = mybir.AluOpType
ACT = mybir.ActivationFunctionType
BF = ml_dtypes.bfloat16

B, C, F = 8, 1024, 4096
P = 128
CT = C // P          # 8 c-tiles
FT = F // P          # 32 f-tiles
NC2 = C // 512       # 2
EPS = 1e-5

S_LAM, S_EU, S_MK, S_MV, S_MR, S_KB, S_VB, S_RB, S_FK, S_FV, S_FR, S_FFK, S_FFR, S_RRB, S_FRR = range(15)
NSLOT = 15
# lncol slots: rstd, negb=-mu*rstd, std
L_RSTD, L_NEGB, L_STD = range(3)


def _bcast_free(col_ap, n):
    """per-partition [128,1] column AP -> [128,n] stride-0 broadcast AP."""
    return bass.AP(tensor=col_ap.tensor, offset=col_ap.offset,
                   ap=[col_ap.ap[0], [0, n]])


def build_nc(T):
    NJ = T // 512
    TT = T // 128
    nc = bacc.Bacc("TRN2", target_bir_lowering=False)

    # ---------------- DRAM I/O ----------------
    x_d = nc.dram_tensor("x", [T, C], BF16, kind="ExternalInput")
    wkT_d = nc.dram_tensor("wkT", [C, C], BF16, kind="ExternalInput")
    wvT_d = nc.dram_tensor("wvT", [C, C], BF16, kind="ExternalInput")
    wrT_d = nc.dram_tensor("wrT", [C, C], BF16, kind="ExternalInput")
    woT_d = nc.dram_tensor("woT", [C, C], BF16, kind="ExternalInput")
    shT_d = nc.dram_tensor("shT", [C, C], BF16, kind="ExternalInput")
    fwkT_d = nc.dram_tensor("fwkT", [C, F], BF16, kind="ExternalInput")
    fwrT_d = nc.dram_tensor("fwrT", [C, C], BF16, kind="ExternalInput")
    fwvT_d = nc.dram_tensor("fwvT", [F, C], BF16, kind="ExternalInput")
    cvec_d = nc.dram_tensor("cvec", [P, NSLOT, CT], F32, kind="ExternalInput")
    fvec_d = nc.dram_tensor("fvec", [P, 2, FT], F32, kind="ExternalInput")
    lncol_d = nc.dram_tensor("lncol", [P, 3, TT], F32, kind="ExternalInput")
    musrow_d = nc.dram_tensor("musrow", [1, T], BF16, kind="ExternalInput")
    srow_d = nc.dram_tensor("srow", [1, C], BF16, kind="ExternalInput")
    ident_d = nc.dram_tensor("ident", [P, P], BF16, kind="ExternalInput")
    out_d = nc.dram_tensor("out", [T, C], F32, kind="ExternalOutput")

    # DRAM scratch (spills for phase 3)
    o1d = nc.dram_tensor("o1d", [T, C], BF16)
    gkTd = nc.dram_tensor("gkTd", [C, T], BF16)
    grTd = nc.dram_tensor("grTd", [C, T], BF16)

    with tile.TileContext(nc) as tc:
        with tc.tile_pool(name="const", bufs=1) as pc, \
             tc.tile_pool(name="psum", bufs=2, space="PSUM") as pp:

            # ---- constants / carries (outer pool, live whole kernel) ----
            cv = pc.tile([P, NSLOT, CT], F32, tag="cv")
            nc.sync.dma_start(out=cv, in_=cvec_d[:, :, :])
            fv = pc.tile([P, 2, FT], F32, tag="fv")
            nc.sync.dma_start(out=fv, in_=fvec_d[:, :, :])
            lncol = pc.tile([P, 3, TT], F32, tag="lncol")
            nc.sync.dma_start(out=lncol, in_=lncol_d[:, :, :])
            musrow = pc.tile([1, T], BF16, tag="musrow")
            nc.sync.dma_start(out=musrow, in_=musrow_d[:, :])
            srow = pc.tile([1, C], BF16, tag="srow")
            nc.sync.dma_start(out=srow, in_=srow_d[:, :])
            ident = pc.tile([P, P], BF16, tag="ident")
            nc.sync.dma_start(out=ident, in_=ident_d[:, :])
            epst = pc.tile([P, 1], F32, tag="epst")
            nc.vector.memset(epst, EPS)
            carA = pc.tile([P, CT], F32, tag="carA")
            carB = pc.tile([P, CT], F32, tag="carB")
            gcar = pc.tile([P, CT, 1], BF16, tag="gcar")

            def cvc(slot, ci):
                return cv[:, slot, ci:ci + 1]

            def lnc(slot, tt):
                return lncol[:, slot, tt:tt + 1]

            # tail pool outlives the big pool (deferred A2b overlaps phase-3
            # weight loads); phase-3 pools live on the right side of SBUF.
            pt_pool = tc.alloc_tile_pool(name="tail", bufs=1)
            pb_pool = tc.alloc_tile_pool(name="big", bufs=1)
            pl = pb_pool
            ptl = pt_pool

            def ldw(pool, tag, dram):
                w = pool.tile([P, CT, C], BF16, tag=tag)
                r = dram[:, :].rearrange("(ci p) co -> p ci co", p=P)
                nc.sync.dma_start(out=w[:, :, 0:512], in_=r[:, :, 0:512])
                nc.sync.dma_start(out=w[:, :, 512:1024], in_=r[:, :, 512:1024])
                return w

            hn_t = {}
            w_sb = {}

            def stage_A1(j, xpre=None):
                """hn = (x-mu)*rstd via Act (host stats) + PE-transpose"""
                hnc = ptl.tile([P, CT, 513], BF16, tag="hnc", bufs=3, name=f"hnc{j}")
                hn_t[j] = hnc
                if j == 0:
                    nc.vector.memset(hnc[:, :, 0:1], 0.0)
                else:
                    nc.gpsimd.tensor_copy(hnc[:, :, 0:1], hn_t[j - 1][:, :, 512:513])
                for tl in range(4):
                    tt = 4 * j + tl
                    if xpre is not None:
                        xt = xpre[:, tl, :]
                    else:
                        xt = pl.tile([P, C], BF16, tag="xin", bufs=2, name=f"xt{tt}")
                        nc.sync.dma_start(out=xt, in_=x_d[tt * P:(tt + 1) * P, :])
                    hnb = ptl.tile([P, C], BF16, tag="hnn", bufs=2, name=f"hnb{tt}")
                    nc.scalar.activation(hnb, xt, ACT.Identity,
                                         bias=lnc(L_NEGB, tt), scale=lnc(L_RSTD, tt))
                    ptr = pp.tile([P, CT, P], BF16, tag="pt", name=f"ptr{tt}")
                    for ci in range(CT):
                        nc.tensor.transpose(ptr[:, ci, :], hnb[:, ci * P:(ci + 1) * P], ident)
                    nc.scalar.activation(hnc[:, :, 1 + tl * P:1 + (tl + 1) * P], ptr, ACT.Copy)

            def stage_MIX(j):
                """time-mixes: xr/xk on DVE stt; xv decomposed on Pool"""
                hnc = hn_t[j]
                xk = pl.tile([P, CT, 512], BF16, tag="xk", bufs=1, name=f"xk{j}")
                xv = pl.tile([P, CT, 512], BF16, tag="xv", bufs=1, name=f"xv{j}")
                xr = pl.tile([P, CT, 512], BF16, tag="xr", bufs=1, name=f"xr{j}")
                for ci in range(CT):
                    d = ptl.tile([P, 512], BF16, tag="mixd", bufs=2, name=f"d{j}_{ci}")
                    nc.gpsimd.tensor_sub(d, hnc[:, ci, 1:513], hnc[:, ci, 0:512])
                    nc.vector.scalar_tensor_tensor(xr[:, ci, :], d, cvc(S_MR, ci), hnc[:, ci, 0:512], AL.mult, AL.add)
                    nc.vector.scalar_tensor_tensor(xk[:, ci, :], d, cvc(S_MK, ci), hnc[:, ci, 0:512], AL.mult, AL.add)
                    nc.gpsimd.tensor_mul(xv[:, ci, :], d, _bcast_free(cvc(S_MV, ci), 512))
                    nc.gpsimd.tensor_add(xv[:, ci, :], xv[:, ci, :], hnc[:, ci, 0:512])
                return xk, xv, xr

            def stage_B1a(j, xr):
                """r matmuls + sigmoid batch"""
                sra = pl.tile([P, CT, 512], BF16, tag="sr", bufs=1, name=f"sra{j}")
                for co in range(CT):
                    pr = pp.tile([P, 512], F32, tag="p2", name=f"pr{j}_{co}")
                    for ci in range(CT):
                        nc.tensor.matmul(pr, wr_sb[:, ci, co * P:(co + 1) * P], xr[:, ci, :],
                                         start=(ci == 0), stop=(ci == CT - 1))
                    if j == 0:
                        nc.vector.tensor_scalar_add(pr[:, 0:1], pr[:, 0:1], cvc(S_FR, co))
                    nc.scalar.activation(sra[:, co, :], pr, ACT.Sigmoid, bias=cvc(S_RB, co))
                return sra

            def stage_B1c(j, xk, xv, sra, fillers=()):
                """k/v matmuls + Exp batch + WKV recurrence; fillers are
                PE work (deferred short-matmul groups) interleaved per co
                to pace k/v production to the DVE consumption rate."""
                rw = ptl.tile([P, CT, 512], BF16, tag="rw", bufs=1, name=f"rw{j}")
                for co in range(CT):
                    if co < len(fillers):
                        fillers[co]()
                    pk = pp.tile([P, 512], F32, tag="p2", name=f"pk{j}_{co}")
                    pv = pp.tile([P, 512], F32, tag="p1", name=f"pv{j}_{co}")
                    for ci in range(CT):
                        nc.tensor.matmul(pk, wk_sb[:, ci, co * P:(co + 1) * P], xk[:, ci, :],
                                         start=(ci == 0), stop=(ci == CT - 1))
                    for ci in range(CT):
                        nc.tensor.matmul(pv, wv_sb[:, ci, co * P:(co + 1) * P], xv[:, ci, :],
                                         start=(ci == 0), stop=(ci == CT - 1))
                    if j == 0:
                        nc.vector.tensor_scalar_add(pk[:, 0:1], pk[:, 0:1], cvc(S_FK, co))
                        nc.vector.tensor_scalar_add(pv[:, 0:1], pv[:, 0:1], cvc(S_FV, co))
                    ek = pl.tile([P, 512], F32, tag="ek", bufs=2, name=f"ek{j}_{co}")
                    nc.scalar.activation(ek, pk, ACT.Exp, bias=cvc(S_KB, co))
                    ekv = pl.tile([P, 512], F32, tag="ekv", bufs=1, name=f"ekv{j}_{co}")
                    nc.vector.scalar_tensor_tensor(ekv, pv, cvc(S_VB, co), ek, AL.add, AL.mult)
                    a_ = pl.tile([P, 513], F32, tag="a", bufs=1, name=f"a{j}_{co}")
                    b_ = pl.tile([P, 513], F32, tag="b", bufs=1, name=f"b{j}_{co}")
                    if j == 0:
                        nc.vector.memset(a_[:, 0:1], 0.0)
                        nc.vector.memset(b_[:, 0:1], 0.0)
                    else:
                        nc.vector.tensor_copy(a_[:, 0:1], carA[:, co:co + 1])
                        nc.vector.tensor_copy(b_[:, 0:1], carB[:, co:co + 1])
                    lam_bc = _bcast_free(cvc(S_LAM, co), 512)
                    nc.vector.tensor_tensor_scan(a_[:, 1:513], lam_bc, ekv, a_[:, 0:1], AL.mult, AL.add)
                    nc.vector.tensor_tensor_scan(b_[:, 1:513], lam_bc, ek, b_[:, 0:1], AL.mult, AL.add)
                    nc.gpsimd.tensor_copy(carA[:, co:co + 1], a_[:, 512:513])
                    nc.gpsimd.tensor_copy(carB[:, co:co + 1], b_[:, 512:513])
                    # num in-place on ekv, den in-place on ek (DVE); muls on Pool
                    nc.vector.scalar_tensor_tensor(ekv, ekv, cvc(S_EU, co), a_[:, 0:512], AL.mult, AL.add)
                    nc.vector.scalar_tensor_tensor(ek, ek, cvc(S_EU, co), b_[:, 0:512], AL.mult, AL.add)
                    nc.vector.reciprocal(ek, ek)
                    nc.gpsimd.tensor_mul(ekv, ekv, ek)
                    nc.gpsimd.tensor_mul(rw[:, co, :], ekv, sra[:, co, :])
                return rw

            def alloc_o1(j):
                return ptl.tile([P, 4, C], BF16, tag="o1", bufs=1, name=f"o1_{j}")

            def make_ps_fillers(j, o1):
                """short (+rank-1) matmul groups as per-co filler closures"""
                hnc = hn_t[j]
                def mk(tl, nco):
                    def emit():
                        tt = 4 * j + tl
                        ps = pp.tile([P, 512], F32, tag="p0", name=f"ps{tt}_{nco}")
                        for ci in range(CT):
                            nc.tensor.matmul(ps, hnc[:, ci, 1 + tl * P:1 + (tl + 1) * P],
                                             sh_sb[:, ci, nco * 512:(nco + 1) * 512],
                                             start=(ci == 0), stop=False)
                        # rank-1: (mu/std)_t * srow_co ; then o1 = psum*std
                        nc.tensor.matmul(ps, musrow[0:1, tt * P:(tt + 1) * P],
                                         srow[0:1, nco * 512:(nco + 1) * 512],
                                         start=False, stop=True)
                        nc.scalar.activation(o1[:, tl, nco * 512:(nco + 1) * 512], ps,
                                             ACT.Identity, scale=lnc(L_STD, tt))
                    return emit
                return [mk(tl, nco) for tl in range(4) for nco in range(NC2)]

            def stage_A2a_wo(j, rw, o1):
                """wo matmuls + o1 assembly + spill"""
                for tl in range(4):
                    tt = 4 * j + tl
                    for nco in range(NC2):
                        pw = pp.tile([P, 512], F32, tag="p0", name=f"pw{tt}_{nco}")
                        for ci in range(CT):
                            nc.tensor.matmul(pw, rw[:, ci, tl * P:(tl + 1) * P],
                                             wo_sb[:, ci, nco * 512:(nco + 1) * 512],
                                             start=(ci == 0), stop=(ci == CT - 1))
                        o1s = o1[:, tl, nco * 512:(nco + 1) * 512]
                        nc.vector.tensor_add(o1s, o1s, pw)
                    nc.sync.dma_start(out=o1d[tt * P:(tt + 1) * P, :], in_=o1[:, tl, :])

            def stage_A2b(j, o1):
                """ln2 + gn transpose + ffn mixes + spills"""
                gnc = ptl.tile([P, CT, 513], BF16, tag="gnc", bufs=1, name=f"gnc{j}")
                if j == 0:
                    nc.vector.memset(gnc[:, :, 0:1], 0.0)
                else:
                    nc.gpsimd.tensor_copy(gnc[:, :, 0:1], gcar[:, :, :])
                for tl in range(4):
                    tt = 4 * j + tl
                    st2 = ptl.tile([P, 2, 6], F32, tag="st", bufs=2, name=f"st2_{tt}")
                    nc.vector.bn_stats(out=st2[:, 0, :], in_=o1[:, tl, 0:512])
                    nc.vector.bn_stats(out=st2[:, 1, :], in_=o1[:, tl, 512:1024])
                    mv2 = ptl.tile([P, 2], F32, tag="mv", bufs=2, name=f"mv2_{tt}")
                    nc.vector.bn_aggr(out=mv2, in_=st2)
                    lv2 = ptl.tile([P, 1], F32, tag="lv", bufs=2, name=f"lv2_{tt}")
                    nc.scalar.activation(lv2, mv2[:, 1:2], ACT.Ln, bias=epst)
                    rstd2 = ptl.tile([P, 1], F32, tag="rstd", bufs=2, name=f"rstd2_{tt}")
                    nc.scalar.activation(rstd2, lv2, ACT.Exp, scale=-0.5)
                    negb2 = ptl.tile([P, 1], F32, tag="negb", bufs=2, name=f"negb2_{tt}")
                    nc.vector.tensor_scalar(negb2, mv2[:, 0:1], rstd2, -1.0, AL.mult, AL.mult)
                    gnb = ptl.tile([P, C], BF16, tag="hnn", bufs=2, name=f"gnb{tt}")
                    nc.scalar.activation(gnb, o1[:, tl, :], ACT.Identity, bias=negb2, scale=rstd2)
                    ptr2 = pp.tile([P, CT, P], BF16, tag="pt", name=f"ptr2_{tt}")
                    for ci in range(CT):
                        nc.tensor.transpose(ptr2[:, ci, :], gnb[:, ci * P:(ci + 1) * P], ident)
                    nc.scalar.activation(gnc[:, :, 1 + tl * P:1 + (tl + 1) * P], ptr2, ACT.Copy)
                nc.gpsimd.tensor_copy(gcar[:, :, :], gnc[:, :, 512:513])
                gk = ptl.tile([P, CT, 512], BF16, tag="gk", bufs=1, name=f"gk{j}")
                gr = ptl.tile([P, CT, 512], BF16, tag="gr", bufs=1, name=f"gr{j}")
                for ci in range(CT):
                    d2 = ptl.tile([P, 512], BF16, tag="mixd", bufs=2, name=f"d2_{j}_{ci}")
                    nc.gpsimd.tensor_sub(d2, gnc[:, ci, 1:513], gnc[:, ci, 0:512])
                    nc.vector.scalar_tensor_tensor(gk[:, ci, :], d2, cvc(S_FFK, ci), gnc[:, ci, 0:512], AL.mult, AL.add)
                    nc.vector.scalar_tensor_tensor(gr[:, ci, :], d2, cvc(S_FFR, ci), gnc[:, ci, 0:512], AL.mult, AL.add)
                nc.sync.dma_start(
                    out=gkTd[:, :].rearrange("(ci p) t -> p ci t", p=P)[:, :, j * 512:(j + 1) * 512],
                    in_=gk)
                nc.sync.dma_start(
                    out=grTd[:, :].rearrange("(ci p) t -> p ci t", p=P)[:, :, j * 512:(j + 1) * 512],
                    in_=gr)

            # ---------------- main loop ----------------
            stage_A1(0)
            mix_t = {0: stage_MIX(0)}
            o1_t = {}
            wr_sb = w_sb["wr"] = ldw(pl, "wr", wrT_d)
            wk_sb = w_sb["wk"] = ldw(pl, "wk", wkT_d)
            wv_sb = w_sb["wv"] = ldw(pl, "wv", wvT_d)
            sh_sb = w_sb["sh"] = ldw(ptl, "sh", shT_d)
            wo_sb = w_sb["wo"] = ldw(ptl, "wo", woT_d)
            rw_t = {}
            for j in range(NJ):
                if j + 1 < NJ:
                    stage_A1(j + 1)
                xk, xv, xr = mix_t.pop(j)
                sra = stage_B1a(j, xr)
                if j >= 1:
                    o1p = alloc_o1(j - 1)
                    fillers = make_ps_fillers(j - 1, o1p)
                else:
                    o1p, fillers = None, ()
                rw_t[j] = stage_B1c(j, xk, xv, sra, fillers)
                if j + 1 < NJ:
                    mix_t[j + 1] = stage_MIX(j + 1)
                if j >= 1:
                    stage_A2a_wo(j - 1, rw_t.pop(j - 1), o1p)
                    stage_A2b(j - 1, o1p)

            pb_pool.release()

            # phase-3 front weights load while the last A2b tail runs
            p3a = tc.alloc_tile_pool(name="p3a", bufs=1, side="right")
            fwr_sb = ldw(p3a, "fwr", fwrT_d)
            fwk_sb = p3a.tile([P, CT, F], BF16, tag="fwk")
            rk = fwkT_d[:, :].rearrange("(ci p) fo -> p ci fo", p=P)
            for q in range(4):
                nc.sync.dma_start(out=fwk_sb[:, :, q * 1024:(q + 1) * 1024],
                                  in_=rk[:, :, q * 1024:(q + 1) * 1024])

            o1_last = alloc_o1(NJ - 1)
            for f in make_ps_fillers(NJ - 1, o1_last):
                f()
            stage_A2a_wo(NJ - 1, rw_t.pop(NJ - 1), o1_last)
            stage_A2b(NJ - 1, o1_last)
            pt_pool.release()

            p3b = tc.alloc_tile_pool(name="p3b", bufs=1, side="right")
            fwv_sb = p3b.tile([P, FT, C], BF16, tag="fwv")
            rv = fwvT_d[:, :].rearrange("(fi p) co -> p fi co", p=P)
            for q in range(4):
                nc.sync.dma_start(out=fwv_sb[:, q * 8:(q + 1) * 8, :],
                                  in_=rv[:, q * 8:(q + 1) * 8, :])

            # ---------------- phase 3: FFN back ----------------
            for j in range(NJ):
                gki = p3a.tile([P, CT, 512], BF16, tag="gki", bufs=1, name=f"gki{j}")
                nc.sync.dma_start(
                    in_=gkTd[:, :].rearrange("(ci p) t -> p ci t", p=P)[:, :, j * 512:(j + 1) * 512],
                    out=gki)
                gri = p3a.tile([P, CT, 512], BF16, tag="gri", bufs=1, name=f"gri{j}")
                nc.sync.dma_start(
                    in_=grTd[:, :].rearrange("(ci p) t -> p ci t", p=P)[:, :, j * 512:(j + 1) * 512],
                    out=gri)
                # fwr matmul + sigmoid + transpose to [t,c]
                s2t = p3b.tile([P, 4, C], BF16, tag="s2t", bufs=1, name=f"s2t{j}")
                for co in range(CT):
                    prr = pp.tile([P, 512], F32, tag="p2", name=f"prr{j}_{co}")
                    for ci in range(CT):
                        nc.tensor.matmul(prr, fwr_sb[:, ci, co * P:(co + 1) * P], gri[:, ci, :],
                                         start=(ci == 0), stop=(ci == CT - 1))
                    if j == 0:
                        nc.vector.tensor_scalar_add(prr[:, 0:1], prr[:, 0:1], cvc(S_FRR, co))
                    sr2 = p3b.tile([P, 512], BF16, tag="sr2", bufs=2, name=f"sr2_{j}_{co}")
                    nc.scalar.activation(sr2, prr, ACT.Sigmoid, bias=cvc(S_RRB, co))
                    ptr3 = pp.tile([P, 4, P], BF16, tag="pt", name=f"ptr3_{j}_{co}")
                    for tl in range(4):
                        nc.tensor.transpose(ptr3[:, tl, :], sr2[:, tl * P:(tl + 1) * P], ident)
                    nc.scalar.activation(s2t[:, :, co * P:(co + 1) * P], ptr3, ACT.Copy)
                # kk2 + kv, in two t-halves of 256
                for h in range(2):
                    kk2 = p3b.tile([P, FT, 256], BF16, tag="kk2", bufs=1, name=f"kk2_{j}_{h}")
                    hs = slice(h * 256, (h + 1) * 256)
                    for ft in range(FT):
                        pkk = pp.tile([P, 256], F32, tag="p0", name=f"pkk{j}_{h}_{ft}")
                        for ci in range(CT):
                            nc.tensor.matmul(pkk, fwk_sb[:, ci, ft * P:(ft + 1) * P], gki[:, ci, hs],
                                             start=(ci == 0), stop=(ci == CT - 1))
                        if j == 0 and h == 0:
                            nc.vector.tensor_scalar_add(pkk[:, 0:1], pkk[:, 0:1], fv[:, 1, ft:ft + 1])
                        kr = p3b.tile([P, 256], BF16, tag="kr", bufs=2, name=f"kr{j}_{h}_{ft}")
                        nc.scalar.activation(kr, pkk, ACT.Relu, bias=fv[:, 0, ft:ft + 1])
                        nc.vector.tensor_mul(kk2[:, ft, :], kr, kr)
                    for tlh in range(2):
                        tl = h * 2 + tlh
                        tt = 4 * j + tl
                        o1r = p3b.tile([P, C], BF16, tag="o1r", bufs=2, name=f"o1r{tt}")
                        nc.sync.dma_start(out=o1r, in_=o1d[tt * P:(tt + 1) * P, :])
                        for nco in range(NC2):
                            pkv = pp.tile([P, 512], F32, tag="p1", name=f"pkv{tt}_{nco}")
                            for ft in range(FT):
                                nc.tensor.matmul(pkv, kk2[:, ft, tlh * P:(tlh + 1) * P],
                                                 fwv_sb[:, ft, nco * 512:(nco + 1) * 512],
                                                 start=(ft == 0), stop=(ft == FT - 1))
                            tmpv = p3b.tile([P, 512], F32, tag="kvt", bufs=2, name=f"kvt{tt}_{nco}")
                            nc.vector.tensor_mul(tmpv, pkv, s2t[:, tl, nco * 512:(nco + 1) * 512])
                            nc.vector.tensor_add(tmpv, tmpv, o1r[:, nco * 512:(nco + 1) * 512])
                            nc.sync.dma_start(out=out_d[tt * P:(tt + 1) * P, nco * 512:(nco + 1) * 512],
                                              in_=tmpv)

            p3b.release()
            p3a.release()

    nc.compile()
    return nc


_NC_CACHE = {}


def get_nc(T):
    if T not in _NC_CACHE:
        _NC_CACHE[T] = build_nc(T)
    return _NC_CACHE[T]


def host_prep(inp, T):
    """Build per-core in_maps from full inputs (float64 math on host)."""
    f8 = lambda a: np.asarray(a, np.float64)
    x = np.asarray(inp["x"], np.float32)
    w1, b1 = f8(inp["ln1_w"]), f8(inp["ln1_b"])
    w2, b2 = f8(inp["ln2_w"]), f8(inp["ln2_b"])
    Wk, Wv, Wr, Wo = f8(inp["att_Wk"]), f8(inp["att_Wv"]), f8(inp["att_Wr"]), f8(inp["att_Wo"])
    Wsh = f8(inp["short_W"])
    fWk, fWr, fWv = f8(inp["ffn_Wk"]), f8(inp["ffn_Wr"]), f8(inp["ffn_Wv"])
    mk, mvx, mr = f8(inp["att_mix_k"]), f8(inp["att_mix_v"]), f8(inp["att_mix_r"])
    fk, fr = f8(inp["ffn_mix_k"]), f8(inp["ffn_mix_r"])
    decay, first = f8(inp["att_time_decay"]), f8(inp["att_time_first"])

    def pack_c(v):
        return np.asarray(v, np.float32).reshape(CT, P).T  # [128, CT]

    lam = np.exp(-np.exp(decay))
    eu = np.exp(first)
    kbias = Wk @ b1
    vbias = Wv @ b1
    rbias = Wr @ b1
    fixk = -Wk @ ((1.0 - mk) * b1)
    fixv = -Wv @ ((1.0 - mvx) * b1)
    fixr = -Wr @ ((1.0 - mr) * b1)
    kkbias = fWk @ b2
    fixkk = -fWk @ ((1.0 - fk) * b2)
    rrbias = fWr @ b2
    fixrr = -fWr @ ((1.0 - fr) * b2)
    srow = Wsh.sum(axis=1)

    cvec = np.stack([pack_c(v) for v in
                     [lam, eu, mk, mvx, mr, kbias, vbias, rbias,
                      fixk, fixv, fixr, fk, fr, rrbias, fixrr]], axis=1)  # [128, 15, 8]
    fvec = np.stack([np.asarray(v, np.float32).reshape(FT, P).T for v in [kkbias, fixkk]],
                    axis=1)  # [128, 2, 32]

    shared = {
        "wkT": np.ascontiguousarray((Wk * w1[None, :]).T.astype(BF)),
        "wvT": np.ascontiguousarray((Wv * w1[None, :]).T.astype(BF)),
        "wrT": np.ascontiguousarray((Wr * w1[None, :]).T.astype(BF)),
        "woT": np.ascontiguousarray(Wo.T.astype(BF)),
        "shT": np.ascontiguousarray(Wsh.T.astype(BF)),
        "fwkT": np.ascontiguousarray((fWk * w2[None, :]).T.astype(BF)),
        "fwrT": np.ascontiguousarray((fWr * w2[None, :]).T.astype(BF)),
        "fwvT": np.ascontiguousarray(fWv.T.astype(BF)),
        "cvec": np.ascontiguousarray(cvec.astype(np.float32)),
        "fvec": np.ascontiguousarray(fvec.astype(np.float32)),
        "srow": np.ascontiguousarray(srow.reshape(1, C).astype(BF)),
        "ident": np.ascontiguousarray(np.eye(P, dtype=np.float32).astype(BF)),
    }
    TTl = T // P
    in_maps = []
    for b in range(x.shape[0]):
        m = dict(shared)
        xb = f8(x[b, :T, :])
        mu = xb.mean(axis=1)
        var = xb.var(axis=1)
        std = np.sqrt(var + EPS)
        rstd = 1.0 / std
        negb = -mu * rstd
        lncol = np.stack([rstd, negb, std], axis=0)          # [3, T]
        lncol = lncol.reshape(3, TTl, P).transpose(2, 0, 1)  # [128, 3, TT]
        m["lncol"] = np.ascontiguousarray(lncol.astype(np.float32))
        m["musrow"] = np.ascontiguousarray((mu * rstd).reshape(1, T).astype(BF))
        m["x"] = np.ascontiguousarray(x[b, :T, :].astype(BF))
        in_maps.append(m)
    return in_maps


def kernel(**inputs):
    T = 2048
    nc = get_nc(T)
    in_maps = host_prep(inputs, T)
    res = run_bass_kernel_spmd(nc, in_maps, core_ids=list(range(len(in_maps))))
    out = np.stack([r["out"] for r in res.results], axis=0)
    return out.astype(np.float32)


# revision 38
# speedup vs baseline: 1.0041x; 1.0041x over previous
"""RWKV-style Block kernel for 8 Trainium2 NeuronCores (batch-parallel SPMD).

v4: software-pipelined main loop with DEFERRED sh/wo matmuls (one iteration
late, so the PE never waits on the current chunk's WKV recurrence), host-side
LN1 statistics (device does only the affine normalize), the short-cut mean
term folded into the matmul as a rank-1 update, PE transposes, batched Act
tables, staged SBUF pools so phase-3 weights load during the last chunk tail.

Iteration j emission (chunks of 512 tokens):
  A1(j+1):  x bf16 -> hn = (x-mu)*rstd (Act, host stats) -> PE-transpose
  MIX(j+1): time-mixes xk/xv/xr
  B1a(j):   r matmuls -> sigmoid batch
  B1c(j):   k/v matmuls -> Exp batch -> WKV scan -> rw
  A2a(j-1): short matmuls (+rank-1 mu/std*srow) -> o1=ps*std; wo matmuls,
            o1 += att; spill o1
  A2b(j-1): ln2 -> gn -> PE-transpose -> ffn mixes -> spill gkT/grT
Phase 3 per chunk: fwr matmul -> sigmoid sr2 -> PE-transpose [t,c];
  kk2 = relu(fwk@gk+b)^2 -> kv matmuls -> out = o1 + sr2*kv
"""
import numpy as np
import ml_dtypes

import concourse.bass as bass
import concourse.bacc as bacc
import concourse.mybir as mybir
import concourse.tile as tile
from concourse.bass_utils import run_bass_kernel_spmd

F32 = mybir.dt.float32
BF16 = mybir.dt.bfloat16
AL = mybir.AluOpType
ACT = mybir.ActivationFunctionType
BF = ml_dtypes.bfloat16

B, C, F = 8, 1024, 4096
P = 128
CT = C // P          # 8 c-tiles
FT = F // P          # 32 f-tiles
NC2 = C // 512       # 2
EPS = 1e-5

S_LAM, S_EU, S_MK, S_MV, S_MR, S_KB, S_VB, S_RB, S_FK, S_FV, S_FR, S_FFK, S_FFR, S_RRB, S_FRR = range(15)
NSLOT = 15
# lncol slots: rstd, negb=-mu*rstd, std
L_RSTD, L_NEGB, L_STD = range(3)


def _bcast_free(col_ap, n):
    """per-partition [128,1] column AP -> [128,n] stride-0 broadcast AP."""
    return bass.AP(tensor=col_ap.tensor, offset=col_ap.offset,
                   ap=[col_ap.ap[0], [0, n]])


def build_nc(T):
    NJ = T // 512
    TT = T // 128
    nc = bacc.Bacc("TRN2", target_bir_lowering=False)

    # ---------------- DRAM I/O ----------------
    x_d = nc.dram_tensor("x", [T, C], BF16, kind="ExternalInput")
    wkT_d = nc.dram_tensor("wkT", [C, C], BF16, kind="ExternalInput")
    wvT_d = nc.dram_tensor("wvT", [C, C], BF16, kind="ExternalInput")
    wrT_d = nc.dram_tensor("wrT", [C, C], BF16, kind="ExternalInput")
    woT_d = nc.dram_tensor("woT", [C, C], BF16, kind="ExternalInput")
    shT_d = nc.dram_tensor("shT", [C, C], BF16, kind="ExternalInput")
    fwkT_d = nc.dram_tensor("fwkT", [C, F], BF16, kind="ExternalInput")
    fwrT_d = nc.dram_tensor("fwrT", [C, C], BF16, kind="ExternalInput")
    fwvT_d = nc.dram_tensor("fwvT", [F, C], BF16, kind="ExternalInput")
    cvec_d = nc.dram_tensor("cvec", [P, NSLOT, CT], F32, kind="ExternalInput")
    fvec_d = nc.dram_tensor("fvec", [P, 2, FT], F32, kind="ExternalInput")
    lncol_d = nc.dram_tensor("lncol", [P, 3, TT], F32, kind="ExternalInput")
    musrow_d = nc.dram_tensor("musrow", [1, T], BF16, kind="ExternalInput")
    srow_d = nc.dram_tensor("srow", [1, C], BF16, kind="ExternalInput")
    ident_d = nc.dram_tensor("ident", [P, P], BF16, kind="ExternalInput")
    out_d = nc.dram_tensor("out", [T, C], F32, kind="ExternalOutput")

    # DRAM scratch (spills for phase 3)
    o1d = nc.dram_tensor("o1d", [T, C], BF16)
    gkTd = nc.dram_tensor("gkTd", [C, T], BF16)
    grTd = nc.dram_tensor("grTd", [C, T], BF16)

    with tile.TileContext(nc) as tc:
        with tc.tile_pool(name="const", bufs=1) as pc, \
             tc.tile_pool(name="psum", bufs=2, space="PSUM") as pp:

            # ---- constants / carries (outer pool, live whole kernel) ----
            cv = pc.tile([P, NSLOT, CT], F32, tag="cv")
            nc.sync.dma_start(out=cv, in_=cvec_d[:, :, :])
            fv = pc.tile([P, 2, FT], F32, tag="fv")
            nc.sync.dma_start(out=fv, in_=fvec_d[:, :, :])
            lncol = pc.tile([P, 3, TT], F32, tag="lncol")
            nc.sync.dma_start(out=lncol, in_=lncol_d[:, :, :])
            musrow = pc.tile([1, T], BF16, tag="musrow")
            nc.sync.dma_start(out=musrow, in_=musrow_d[:, :])
            srow = pc.tile([1, C], BF16, tag="srow")
            nc.sync.dma_start(out=srow, in_=srow_d[:, :])
            ident = pc.tile([P, P], BF16, tag="ident")
            nc.sync.dma_start(out=ident, in_=ident_d[:, :])
            epst = pc.tile([P, 1], F32, tag="epst")
            nc.vector.memset(epst, EPS)
            carA = pc.tile([P, CT], F32, tag="carA")
            carB = pc.tile([P, CT], F32, tag="carB")
            gcar = pc.tile([P, CT, 1], BF16, tag="gcar")

            def cvc(slot, ci):
                return cv[:, slot, ci:ci + 1]

            def lnc(slot, tt):
                return lncol[:, slot, tt:tt + 1]

            # tail pool outlives the big pool (deferred A2b overlaps phase-3
            # weight loads); phase-3 pools live on the right side of SBUF.
            pt_pool = tc.alloc_tile_pool(name="tail", bufs=1)
            pb_pool = tc.alloc_tile_pool(name="big", bufs=1)
            pl = pb_pool
            ptl = pt_pool

            def ldw(pool, tag, dram):
                w = pool.tile([P, CT, C], BF16, tag=tag)
                r = dram[:, :].rearrange("(ci p) co -> p ci co", p=P)
                nc.sync.dma_start(out=w[:, :, 0:512], in_=r[:, :, 0:512])
                nc.sync.dma_start(out=w[:, :, 512:1024], in_=r[:, :, 512:1024])
                return w

            hn_t = {}
            w_sb = {}

            def stage_A1(j, xpre=None):
                """hn = (x-mu)*rstd via Act (host stats) + PE-transpose"""
                hnc = ptl.tile([P, CT, 513], BF16, tag="hnc", bufs=3, name=f"hnc{j}")
                hn_t[j] = hnc
                if j == 0:
                    nc.vector.memset(hnc[:, :, 0:1], 0.0)
                else:
                    nc.gpsimd.tensor_copy(hnc[:, :, 0:1], hn_t[j - 1][:, :, 512:513])
                for tl in range(4):
                    tt = 4 * j + tl
                    if xpre is not None:
                        xt = xpre[:, tl, :]
                    else:
                        xt = pl.tile([P, C], BF16, tag="xin", bufs=2, name=f"xt{tt}")
                        nc.sync.dma_start(out=xt, in_=x_d[tt * P:(tt + 1) * P, :])
                    hnb = ptl.tile([P, C], BF16, tag="hnn", bufs=2, name=f"hnb{tt}")
                    nc.scalar.activation(hnb, xt, ACT.Identity,
                                         bias=lnc(L_NEGB, tt), scale=lnc(L_RSTD, tt))
                    ptr = pp.tile([P, CT, P], BF16, tag="pt", name=f"ptr{tt}")
                    for ci in range(CT):
                        nc.tensor.transpose(ptr[:, ci, :], hnb[:, ci * P:(ci + 1) * P], ident)
                    nc.scalar.activation(hnc[:, :, 1 + tl * P:1 + (tl + 1) * P], ptr, ACT.Copy)

            def stage_MIX(j):
                """time-mixes: xr/xk on DVE stt; xv decomposed on Pool"""
                hnc = hn_t[j]
                xk = pl.tile([P, CT, 512], BF16, tag="xk", bufs=1, name=f"xk{j}")
                xv = pl.tile([P, CT, 512], BF16, tag="xv", bufs=1, name=f"xv{j}")
                xr = pl.tile([P, CT, 512], BF16, tag="xr", bufs=1, name=f"xr{j}")
                for ci in range(CT):
                    d = ptl.tile([P, 512], BF16, tag="mixd", bufs=2, name=f"d{j}_{ci}")
                    nc.gpsimd.tensor_sub(d, hnc[:, ci, 1:513], hnc[:, ci, 0:512])
                    nc.vector.scalar_tensor_tensor(xr[:, ci, :], d, cvc(S_MR, ci), hnc[:, ci, 0:512], AL.mult, AL.add)
                    nc.vector.scalar_tensor_tensor(xk[:, ci, :], d, cvc(S_MK, ci), hnc[:, ci, 0:512], AL.mult, AL.add)
                    nc.gpsimd.tensor_mul(xv[:, ci, :], d, _bcast_free(cvc(S_MV, ci), 512))
                    nc.gpsimd.tensor_add(xv[:, ci, :], xv[:, ci, :], hnc[:, ci, 0:512])
                return xk, xv, xr

            def stage_B1a(j, xr):
                """r matmuls + sigmoid batch"""
                sra = pl.tile([P, CT, 512], BF16, tag="sr", bufs=1, name=f"sra{j}")
                for co in range(CT):
                    pr = pp.tile([P, 512], F32, tag="p2", name=f"pr{j}_{co}")
                    for ci in range(CT):
                        nc.tensor.matmul(pr, wr_sb[:, ci, co * P:(co + 1) * P], xr[:, ci, :],
                                         start=(ci == 0), stop=(ci == CT - 1))
                    if j == 0:
                        nc.vector.tensor_scalar_add(pr[:, 0:1], pr[:, 0:1], cvc(S_FR, co))
                    nc.scalar.activation(sra[:, co, :], pr, ACT.Sigmoid, bias=cvc(S_RB, co))
                return sra

            def stage_B1c(j, xk, xv, sra, fillers=()):
                """k/v matmuls + Exp batch + WKV recurrence; fillers are
                PE work (deferred short-matmul groups) interleaved per co
                to pace k/v production to the DVE consumption rate."""
                rw = ptl.tile([P, CT, 512], BF16, tag="rw", bufs=1, name=f"rw{j}")
                for co in range(CT):
                    if co < len(fillers):
                        fillers[co]()
                    pk = pp.tile([P, 512], F32, tag="p2", name=f"pk{j}_{co}")
                    pv = pp.tile([P, 512], F32, tag="p1", name=f"pv{j}_{co}")
                    for ci in range(CT):
                        nc.tensor.matmul(pk, wk_sb[:, ci, co * P:(co + 1) * P], xk[:, ci, :],
                                         start=(ci == 0), stop=(ci == CT - 1))
                    for ci in range(CT):
                        nc.tensor.matmul(pv, wv_sb[:, ci, co * P:(co + 1) * P], xv[:, ci, :],
                                         start=(ci == 0), stop=(ci == CT - 1))
                    if j == 0:
                        nc.vector.tensor_scalar_add(pk[:, 0:1], pk[:, 0:1], cvc(S_FK, co))
                        nc.vector.tensor_scalar_add(pv[:, 0:1], pv[:, 0:1], cvc(S_FV, co))
                    ek = pl.tile([P, 512], F32, tag="ek", bufs=2, name=f"ek{j}_{co}")
                    nc.scalar.activation(ek, pk, ACT.Exp, bias=cvc(S_KB, co))
                    ekv = pl.tile([P, 512], F32, tag="ekv", bufs=1, name=f"ekv{j}_{co}")
                    nc.vector.scalar_tensor_tensor(ekv, pv, cvc(S_VB, co), ek, AL.add, AL.mult)
                    a_ = pl.tile([P, 513], F32, tag="a", bufs=1, name=f"a{j}_{co}")
                    b_ = pl.tile([P, 513], F32, tag="b", bufs=1, name=f"b{j}_{co}")
                    if j == 0:
                        nc.vector.memset(a_[:, 0:1], 0.0)
                        nc.vector.memset(b_[:, 0:1], 0.0)
                    else:
                        nc.vector.tensor_copy(a_[:, 0:1], carA[:, co:co + 1])
                        nc.vector.tensor_copy(b_[:, 0:1], carB[:, co:co + 1])
                    lam_bc = _bcast_free(cvc(S_LAM, co), 512)
                    nc.vector.tensor_tensor_scan(a_[:, 1:513], lam_bc, ekv, a_[:, 0:1], AL.mult, AL.add)
                    nc.vector.tensor_tensor_scan(b_[:, 1:513], lam_bc, ek, b_[:, 0:1], AL.mult, AL.add)
                    nc.gpsimd.tensor_copy(carA[:, co:co + 1], a_[:, 512:513])
                    nc.gpsimd.tensor_copy(carB[:, co:co + 1], b_[:, 512:513])
                    # num in-place on ekv, den in-place on ek (DVE); muls on Pool
                    nc.vector.scalar_tensor_tensor(ekv, ekv, cvc(S_EU, co), a_[:, 0:512], AL.mult, AL.add)
                    nc.vector.scalar_tensor_tensor(ek, ek, cvc(S_EU, co), b_[:, 0:512], AL.mult, AL.add)
                    nc.vector.reciprocal(ek, ek)
                    nc.gpsimd.tensor_mul(ekv, ekv, ek)
                    nc.gpsimd.tensor_mul(rw[:, co, :], ekv, sra[:, co, :])
                return rw

            def alloc_o1(j):
                return ptl.tile([P, 4, C], BF16, tag="o1", bufs=1, name=f"o1_{j}")

            def make_ps_fillers(j, o1):
                """short (+rank-1) matmul groups as per-co filler closures"""
                hnc = hn_t[j]
                def mk(tl, nco):
                    def emit():
                        tt = 4 * j + tl
                        ps = pp.tile([P, 512], F32, tag="p0", name=f"ps{tt}_{nco}")
                        for ci in range(CT):
                            nc.tensor.matmul(ps, hnc[:, ci, 1 + tl * P:1 + (tl + 1) * P],
                                             sh_sb[:, ci, nco * 512:(nco + 1) * 512],
                                             start=(ci == 0), stop=False)
                        # rank-1: (mu/std)_t * srow_co ; then o1 = psum*std
                        nc.tensor.matmul(ps, musrow[0:1, tt * P:(tt + 1) * P],
                                         srow[0:1, nco * 512:(nco + 1) * 512],
                                         start=False, stop=True)
                        nc.scalar.activation(o1[:, tl, nco * 512:(nco + 1) * 512], ps,
                                             ACT.Identity, scale=lnc(L_STD, tt))
                    return emit
                return [mk(tl, nco) for tl in range(4) for nco in range(NC2)]

            def stage_A2a_wo(j, rw, o1):
                """wo matmuls + o1 assembly + spill"""
                for tl in range(4):
                    tt = 4 * j + tl
                    for nco in range(NC2):
                        pw = pp.tile([P, 512], F32, tag="p0", name=f"pw{tt}_{nco}")
                        for ci in range(CT):
                            nc.tensor.matmul(pw, rw[:, ci, tl * P:(tl + 1) * P],
                                             wo_sb[:, ci, nco * 512:(nco + 1) * 512],
                                             start=(ci == 0), stop=(ci == CT - 1))
                        o1s = o1[:, tl, nco * 512:(nco + 1) * 512]
                        nc.vector.tensor_add(o1s, o1s, pw)
                    nc.sync.dma_start(out=o1d[tt * P:(tt + 1) * P, :], in_=o1[:, tl, :])

            def stage_A2b(j, o1):
                """ln2 + gn transpose + ffn mixes + spills"""
                gnc = ptl.tile([P, CT, 513], BF16, tag="gnc", bufs=1, name=f"gnc{j}")
                if j == 0:
                    nc.vector.memset(gnc[:, :, 0:1], 0.0)
                else:
                    nc.gpsimd.tensor_copy(gnc[:, :, 0:1], gcar[:, :, :])
                for tl in range(4):
                    tt = 4 * j + tl
                    st2 = ptl.tile([P, 2, 6], F32, tag="st", bufs=2, name=f"st2_{tt}")
                    nc.vector.bn_stats(out=st2[:, 0, :], in_=o1[:, tl, 0:512])
                    nc.vector.bn_stats(out=st2[:, 1, :], in_=o1[:, tl, 512:1024])
                    mv2 = ptl.tile([P, 2], F32, tag="mv", bufs=2, name=f"mv2_{tt}")
                    nc.vector.bn_aggr(out=mv2, in_=st2)
                    lv2 = ptl.tile([P, 1], F32, tag="lv", bufs=2, name=f"lv2_{tt}")
                    nc.scalar.activation(lv2, mv2[:, 1:2], ACT.Ln, bias=epst)
                    rstd2 = ptl.tile([P, 1], F32, tag="rstd", bufs=2, name=f"rstd2_{tt}")
                    nc.scalar.activation(rstd2, lv2, ACT.Exp, scale=-0.5)
                    negb2 = ptl.tile([P, 1], F32, tag="negb", bufs=2, name=f"negb2_{tt}")
                    nc.vector.tensor_scalar(negb2, mv2[:, 0:1], rstd2, -1.0, AL.mult, AL.mult)
                    gnb = ptl.tile([P, C], BF16, tag="hnn", bufs=2, name=f"gnb{tt}")
                    nc.scalar.activation(gnb, o1[:, tl, :], ACT.Identity, bias=negb2, scale=rstd2)
                    ptr2 = pp.tile([P, CT, P], BF16, tag="pt", name=f"ptr2_{tt}")
                    for ci in range(CT):
                        nc.tensor.transpose(ptr2[:, ci, :], gnb[:, ci * P:(ci + 1) * P], ident)
                    nc.scalar.activation(gnc[:, :, 1 + tl * P:1 + (tl + 1) * P], ptr2, ACT.Copy)
                nc.gpsimd.tensor_copy(gcar[:, :, :], gnc[:, :, 512:513])
                gk = ptl.tile([P, CT, 512], BF16, tag="gk", bufs=1, name=f"gk{j}")
                gr = ptl.tile([P, CT, 512], BF16, tag="gr", bufs=1, name=f"gr{j}")
                for ci in range(CT):
                    d2 = ptl.tile([P, 512], BF16, tag="mixd", bufs=2, name=f"d2_{j}_{ci}")
                    nc.gpsimd.tensor_sub(d2, gnc[:, ci, 1:513], gnc[:, ci, 0:512])
                    nc.vector.scalar_tensor_tensor(gk[:, ci, :], d2, cvc(S_FFK, ci), gnc[:, ci, 0:512], AL.mult, AL.add)
                    nc.vector.scalar_tensor_tensor(gr[:, ci, :], d2, cvc(S_FFR, ci), gnc[:, ci, 0:512], AL.mult, AL.add)
                nc.sync.dma_start(
                    out=gkTd[:, :].rearrange("(ci p) t -> p ci t", p=P)[:, :, j * 512:(j + 1) * 512],
                    in_=gk)
                nc.sync.dma_start(
                    out=grTd[:, :].rearrange("(ci p) t -> p ci t", p=P)[:, :, j * 512:(j + 1) * 512],
                    in_=gr)

            # ---------------- main loop ----------------
            stage_A1(0)
            mix_t = {0: stage_MIX(0)}
            o1_t = {}
            wr_sb = w_sb["wr"] = ldw(pl, "wr", wrT_d)
            wk_sb = w_sb["wk"] = ldw(pl, "wk", wkT_d)
            wv_sb = w_sb["wv"] = ldw(pl, "wv", wvT_d)
            sh_sb = w_sb["sh"] = ldw(ptl, "sh", shT_d)
            wo_sb = w_sb["wo"] = ldw(ptl, "wo", woT_d)
            rw_t = {}
            for j in range(NJ):
                if j + 1 < NJ:
                    stage_A1(j + 1)
                xk, xv, xr = mix_t.pop(j)
                sra = stage_B1a(j, xr)
                if j >= 1:
                    o1p = alloc_o1(j - 1)
                    fillers = make_ps_fillers(j - 1, o1p)
                else:
                    o1p, fillers = None, ()
                rw_t[j] = stage_B1c(j, xk, xv, sra, fillers)
                if j + 1 < NJ:
                    mix_t[j + 1] = stage_MIX(j + 1)
                if j == NJ - 1:
                    # last chunk: big pool is dead after B1c; release it now so
                    # the phase-3 weight DMAs overlap the remaining tail work
                    pb_pool.release()
                    p3a = tc.alloc_tile_pool(name="p3a", bufs=1, side="right")
                    fwr_sb = ldw(p3a, "fwr", fwrT_d)
                    fwk_sb = p3a.tile([P, CT, F], BF16, tag="fwk")
                    rk = fwkT_d[:, :].rearrange("(ci p) fo -> p ci fo", p=P)
                    for q in range(4):
                        nc.sync.dma_start(out=fwk_sb[:, :, q * 1024:(q + 1) * 1024],
                                          in_=rk[:, :, q * 1024:(q + 1) * 1024])
                if j >= 1:
                    stage_A2a_wo(j - 1, rw_t.pop(j - 1), o1p)
                    stage_A2b(j - 1, o1p)

            o1_last = alloc_o1(NJ - 1)
            for f in make_ps_fillers(NJ - 1, o1_last):
                f()
            stage_A2a_wo(NJ - 1, rw_t.pop(NJ - 1), o1_last)
            stage_A2b(NJ - 1, o1_last)
            pt_pool.release()

            p3b = tc.alloc_tile_pool(name="p3b", bufs=1, side="right")
            fwv_sb = p3b.tile([P, FT, C], BF16, tag="fwv")
            rv = fwvT_d[:, :].rearrange("(fi p) co -> p fi co", p=P)
            for q in range(4):
                nc.sync.dma_start(out=fwv_sb[:, q * 8:(q + 1) * 8, :],
                                  in_=rv[:, q * 8:(q + 1) * 8, :])

            # ---------------- phase 3: FFN back ----------------
            for j in range(NJ):
                gki = p3a.tile([P, CT, 512], BF16, tag="gki", bufs=1, name=f"gki{j}")
                nc.sync.dma_start(
                    in_=gkTd[:, :].rearrange("(ci p) t -> p ci t", p=P)[:, :, j * 512:(j + 1) * 512],
                    out=gki)
                gri = p3a.tile([P, CT, 512], BF16, tag="gri", bufs=1, name=f"gri{j}")
                nc.sync.dma_start(
                    in_=grTd[:, :].rearrange("(ci p) t -> p ci t", p=P)[:, :, j * 512:(j + 1) * 512],
                    out=gri)
                # fwr matmul + sigmoid + transpose to [t,c]
                s2t = p3b.tile([P, 4, C], BF16, tag="s2t", bufs=1, name=f"s2t{j}")
                for co in range(CT):
                    prr = pp.tile([P, 512], F32, tag="p2", name=f"prr{j}_{co}")
                    for ci in range(CT):
                        nc.tensor.matmul(prr, fwr_sb[:, ci, co * P:(co + 1) * P], gri[:, ci, :],
                                         start=(ci == 0), stop=(ci == CT - 1))
                    if j == 0:
                        nc.vector.tensor_scalar_add(prr[:, 0:1], prr[:, 0:1], cvc(S_FRR, co))
                    sr2 = p3b.tile([P, 512], BF16, tag="sr2", bufs=2, name=f"sr2_{j}_{co}")
                    nc.scalar.activation(sr2, prr, ACT.Sigmoid, bias=cvc(S_RRB, co))
                    ptr3 = pp.tile([P, 4, P], BF16, tag="pt", name=f"ptr3_{j}_{co}")
                    for tl in range(4):
                        nc.tensor.transpose(ptr3[:, tl, :], sr2[:, tl * P:(tl + 1) * P], ident)
                    nc.scalar.activation(s2t[:, :, co * P:(co + 1) * P], ptr3, ACT.Copy)
                # kk2 + kv, in two t-halves of 256
                for h in range(2):
                    kk2 = p3b.tile([P, FT, 256], BF16, tag="kk2", bufs=1, name=f"kk2_{j}_{h}")
                    hs = slice(h * 256, (h + 1) * 256)
                    for ft in range(FT):
                        pkk = pp.tile([P, 256], F32, tag="p0", name=f"pkk{j}_{h}_{ft}")
                        for ci in range(CT):
                            nc.tensor.matmul(pkk, fwk_sb[:, ci, ft * P:(ft + 1) * P], gki[:, ci, hs],
                                             start=(ci == 0), stop=(ci == CT - 1))
                        if j == 0 and h == 0:
                            nc.vector.tensor_scalar_add(pkk[:, 0:1], pkk[:, 0:1], fv[:, 1, ft:ft + 1])
                        kr = p3b.tile([P, 256], BF16, tag="kr", bufs=2, name=f"kr{j}_{h}_{ft}")
                        nc.scalar.activation(kr, pkk, ACT.Relu, bias=fv[:, 0, ft:ft + 1])
                        nc.vector.tensor_mul(kk2[:, ft, :], kr, kr)
                    for tlh in range(2):
                        tl = h * 2 + tlh
                        tt = 4 * j + tl
                        o1r = p3b.tile([P, C], BF16, tag="o1r", bufs=2, name=f"o1r{tt}")
                        nc.sync.dma_start(out=o1r, in_=o1d[tt * P:(tt + 1) * P, :])
                        for nco in range(NC2):
                            pkv = pp.tile([P, 512], F32, tag="p1", name=f"pkv{tt}_{nco}")
                            for ft in range(FT):
                                nc.tensor.matmul(pkv, kk2[:, ft, tlh * P:(tlh + 1) * P],
                                                 fwv_sb[:, ft, nco * 512:(nco + 1) * 512],
                                                 start=(ft == 0), stop=(ft == FT - 1))
                            tmpv = p3b.tile([P, 512], F32, tag="kvt", bufs=2, name=f"kvt{tt}_{nco}")
                            nc.vector.tensor_mul(tmpv, pkv, s2t[:, tl, nco * 512:(nco + 1) * 512])
                            nc.vector.tensor_add(tmpv, tmpv, o1r[:, nco * 512:(nco + 1) * 512])
                            nc.sync.dma_start(out=out_d[tt * P:(tt + 1) * P, nco * 512:(nco + 1) * 512],
                                              in_=tmpv)

            p3b.release()
            p3a.release()

    nc.compile()
    return nc


_NC_CACHE = {}


def get_nc(T):
    if T not in _NC_CACHE:
        _NC_CACHE[T] = build_nc(T)
    return _NC_CACHE[T]


def host_prep(inp, T):
    """Build per-core in_maps from full inputs (float64 math on host)."""
    f8 = lambda a: np.asarray(a, np.float64)
    x = np.asarray(inp["x"], np.float32)
    w1, b1 = f8(inp["ln1_w"]), f8(inp["ln1_b"])
    w2, b2 = f8(inp["ln2_w"]), f8(inp["ln2_b"])
    Wk, Wv, Wr, Wo = f8(inp["att_Wk"]), f8(inp["att_Wv"]), f8(inp["att_Wr"]), f8(inp["att_Wo"])
    Wsh = f8(inp["short_W"])
    fWk, fWr, fWv = f8(inp["ffn_Wk"]), f8(inp["ffn_Wr"]), f8(inp["ffn_Wv"])
    mk, mvx, mr = f8(inp["att_mix_k"]), f8(inp["att_mix_v"]), f8(inp["att_mix_r"])
    fk, fr = f8(inp["ffn_mix_k"]), f8(inp["ffn_mix_r"])
    decay, first = f8(inp["att_time_decay"]), f8(inp["att_time_first"])

    def pack_c(v):
        return np.asarray(v, np.float32).reshape(CT, P).T  # [128, CT]

    lam = np.exp(-np.exp(decay))
    eu = np.exp(first)
    kbias = Wk @ b1
    vbias = Wv @ b1
    rbias = Wr @ b1
    fixk = -Wk @ ((1.0 - mk) * b1)
    fixv = -Wv @ ((1.0 - mvx) * b1)
    fixr = -Wr @ ((1.0 - mr) * b1)
    kkbias = fWk @ b2
    fixkk = -fWk @ ((1.0 - fk) * b2)
    rrbias = fWr @ b2
    fixrr = -fWr @ ((1.0 - fr) * b2)
    srow = Wsh.sum(axis=1)

    cvec = np.stack([pack_c(v) for v in
                     [lam, eu, mk, mvx, mr, kbias, vbias, rbias,
                      fixk, fixv, fixr, fk, fr, rrbias, fixrr]], axis=1)  # [128, 15, 8]
    fvec = np.stack([np.asarray(v, np.float32).reshape(FT, P).T for v in [kkbias, fixkk]],
                    axis=1)  # [128, 2, 32]

    shared = {
        "wkT": np.ascontiguousarray((Wk * w1[None, :]).T.astype(BF)),
        "wvT": np.ascontiguousarray((Wv * w1[None, :]).T.astype(BF)),
        "wrT": np.ascontiguousarray((Wr * w1[None, :]).T.astype(BF)),
        "woT": np.ascontiguousarray(Wo.T.astype(BF)),
        "shT": np.ascontiguousarray(Wsh.T.astype(BF)),
        "fwkT": np.ascontiguousarray((fWk * w2[None, :]).T.astype(BF)),
        "fwrT": np.ascontiguousarray((fWr * w2[None, :]).T.astype(BF)),
        "fwvT": np.ascontiguousarray(fWv.T.astype(BF)),
        "cvec": np.ascontiguousarray(cvec.astype(np.float32)),
        "fvec": np.ascontiguousarray(fvec.astype(np.float32)),
        "srow": np.ascontiguousarray(srow.reshape(1, C).astype(BF)),
        "ident": np.ascontiguousarray(np.eye(P, dtype=np.float32).astype(BF)),
    }
    TTl = T // P
    in_maps = []
    for b in range(x.shape[0]):
        m = dict(shared)
        xb = f8(x[b, :T, :])
        mu = xb.mean(axis=1)
        var = xb.var(axis=1)
        std = np.sqrt(var + EPS)
        rstd = 1.0 / std
        negb = -mu * rstd
        lncol = np.stack([rstd, negb, std], axis=0)          # [3, T]
        lncol = lncol.reshape(3, TTl, P).transpose(2, 0, 1)  # [128, 3, TT]
        m["lncol"] = np.ascontiguousarray(lncol.astype(np.float32))
        m["musrow"] = np.ascontiguousarray((mu * rstd).reshape(1, T).astype(BF))
        m["x"] = np.ascontiguousarray(x[b, :T, :].astype(BF))
        in_maps.append(m)
    return in_maps


def kernel(**inputs):
    T = 2048
    nc = get_nc(T)
    in_maps = host_prep(inputs, T)
    res = run_bass_kernel_spmd(nc, in_maps, core_ids=list(range(len(in_maps))))
    out = np.stack([r["out"] for r in res.results], axis=0)
    return out.astype(np.float32)


# revision 41
# speedup vs baseline: 1.0096x; 1.0055x over previous
"""RWKV-style Block kernel for 8 Trainium2 NeuronCores (batch-parallel SPMD).

v4: software-pipelined main loop with DEFERRED sh/wo matmuls (one iteration
late, so the PE never waits on the current chunk's WKV recurrence), host-side
LN1 statistics (device does only the affine normalize), the short-cut mean
term folded into the matmul as a rank-1 update, PE transposes, batched Act
tables, staged SBUF pools so phase-3 weights load during the last chunk tail.

Iteration j emission (chunks of 512 tokens):
  A1(j+1):  x bf16 -> hn = (x-mu)*rstd (Act, host stats) -> PE-transpose
  MIX(j+1): time-mixes xk/xv/xr
  B1a(j):   r matmuls -> sigmoid batch
  B1c(j):   k/v matmuls -> Exp batch -> WKV scan -> rw
  A2a(j-1): short matmuls (+rank-1 mu/std*srow) -> o1=ps*std; wo matmuls,
            o1 += att; spill o1
  A2b(j-1): ln2 -> gn -> PE-transpose -> ffn mixes -> spill gkT/grT
Phase 3 per chunk: fwr matmul -> sigmoid sr2 -> PE-transpose [t,c];
  kk2 = relu(fwk@gk+b)^2 -> kv matmuls -> out = o1 + sr2*kv
"""
import numpy as np
import ml_dtypes

import concourse.bass as bass
import concourse.bacc as bacc
import concourse.mybir as mybir
import concourse.tile as tile
from concourse.bass_utils import run_bass_kernel_spmd

F32 = mybir.dt.float32
BF16 = mybir.dt.bfloat16
AL = mybir.AluOpType
ACT = mybir.ActivationFunctionType
BF = ml_dtypes.bfloat16

B, C, F = 8, 1024, 4096
P = 128
CT = C // P          # 8 c-tiles
FT = F // P          # 32 f-tiles
NC2 = C // 512       # 2
EPS = 1e-5

S_LAM, S_EU, S_MK, S_MV, S_MR, S_KB, S_VB, S_RB, S_FK, S_FV, S_FR, S_FFK, S_FFR, S_RRB, S_FRR = range(15)
NSLOT = 15
# lncol slots: rstd, negb=-mu*rstd, std
L_RSTD, L_NEGB, L_STD = range(3)


def _bcast_free(col_ap, n):
    """per-partition [128,1] column AP -> [128,n] stride-0 broadcast AP."""
    return bass.AP(tensor=col_ap.tensor, offset=col_ap.offset,
                   ap=[col_ap.ap[0], [0, n]])


def build_nc(T):
    NJ = T // 512
    TT = T // 128
    nc = bacc.Bacc("TRN2", target_bir_lowering=False)

    # ---------------- DRAM I/O ----------------
    x_d = nc.dram_tensor("x", [T, C], BF16, kind="ExternalInput")
    wkT_d = nc.dram_tensor("wkT", [C, C], BF16, kind="ExternalInput")
    wvT_d = nc.dram_tensor("wvT", [C, C], BF16, kind="ExternalInput")
    wrT_d = nc.dram_tensor("wrT", [C, C], BF16, kind="ExternalInput")
    woT_d = nc.dram_tensor("woT", [C, C], BF16, kind="ExternalInput")
    shT_d = nc.dram_tensor("shT", [C, C], BF16, kind="ExternalInput")
    fwkT_d = nc.dram_tensor("fwkT", [C, F], BF16, kind="ExternalInput")
    fwrT_d = nc.dram_tensor("fwrT", [C, C], BF16, kind="ExternalInput")
    fwvT_d = nc.dram_tensor("fwvT", [F, C], BF16, kind="ExternalInput")
    cvec_d = nc.dram_tensor("cvec", [P, NSLOT, CT], F32, kind="ExternalInput")
    fvec_d = nc.dram_tensor("fvec", [P, 2, FT], F32, kind="ExternalInput")
    lncol_d = nc.dram_tensor("lncol", [P, 3, TT], F32, kind="ExternalInput")
    musrow_d = nc.dram_tensor("musrow", [1, T], BF16, kind="ExternalInput")
    srow_d = nc.dram_tensor("srow", [1, C], BF16, kind="ExternalInput")
    ident_d = nc.dram_tensor("ident", [P, P], BF16, kind="ExternalInput")
    out_d = nc.dram_tensor("out", [T, C], F32, kind="ExternalOutput")

    # DRAM scratch (spills for phase 3)
    o1d = nc.dram_tensor("o1d", [T, C], BF16)
    gkTd = nc.dram_tensor("gkTd", [C, T], BF16)
    grTd = nc.dram_tensor("grTd", [C, T], BF16)

    with tile.TileContext(nc) as tc:
        with tc.tile_pool(name="const", bufs=1) as pc, \
             tc.tile_pool(name="psum", bufs=2, space="PSUM") as pp:

            # ---- constants / carries (outer pool, live whole kernel) ----
            cv = pc.tile([P, NSLOT, CT], F32, tag="cv")
            nc.sync.dma_start(out=cv, in_=cvec_d[:, :, :])
            fv = pc.tile([P, 2, FT], F32, tag="fv")
            nc.sync.dma_start(out=fv, in_=fvec_d[:, :, :])
            lncol = pc.tile([P, 3, TT], F32, tag="lncol")
            nc.sync.dma_start(out=lncol, in_=lncol_d[:, :, :])
            musrow = pc.tile([1, T], BF16, tag="musrow")
            nc.sync.dma_start(out=musrow, in_=musrow_d[:, :])
            srow = pc.tile([1, C], BF16, tag="srow")
            nc.sync.dma_start(out=srow, in_=srow_d[:, :])
            ident = pc.tile([P, P], BF16, tag="ident")
            nc.sync.dma_start(out=ident, in_=ident_d[:, :])
            epst = pc.tile([P, 1], F32, tag="epst")
            nc.vector.memset(epst, EPS)
            carA = pc.tile([P, CT], F32, tag="carA")
            carB = pc.tile([P, CT], F32, tag="carB")
            gcar = pc.tile([P, CT, 1], BF16, tag="gcar")

            def cvc(slot, ci):
                return cv[:, slot, ci:ci + 1]

            def lnc(slot, tt):
                return lncol[:, slot, tt:tt + 1]

            # tail pool outlives the big pool (deferred A2b overlaps phase-3
            # weight loads); phase-3 pools live on the right side of SBUF.
            pt_pool = tc.alloc_tile_pool(name="tail", bufs=1)
            pb_pool = tc.alloc_tile_pool(name="big", bufs=1)
            pl = pb_pool
            ptl = pt_pool

            def ldw(pool, tag, dram):
                w = pool.tile([P, CT, C], BF16, tag=tag)
                r = dram[:, :].rearrange("(ci p) co -> p ci co", p=P)
                nc.sync.dma_start(out=w[:, :, 0:512], in_=r[:, :, 0:512])
                nc.sync.dma_start(out=w[:, :, 512:1024], in_=r[:, :, 512:1024])
                return w

            hn_t = {}
            w_sb = {}

            def stage_A1(j, xpre=None):
                """hn = (x-mu)*rstd via Act (host stats) + PE-transpose"""
                hnc = ptl.tile([P, CT, 513], BF16, tag="hnc", bufs=3, name=f"hnc{j}")
                hn_t[j] = hnc
                if j == 0:
                    nc.vector.memset(hnc[:, :, 0:1], 0.0)
                else:
                    nc.gpsimd.tensor_copy(hnc[:, :, 0:1], hn_t[j - 1][:, :, 512:513])
                for tl in range(4):
                    tt = 4 * j + tl
                    if xpre is not None:
                        xt = xpre[:, tl, :]
                    else:
                        xt = pl.tile([P, C], BF16, tag="xin", bufs=2, name=f"xt{tt}")
                        nc.sync.dma_start(out=xt, in_=x_d[tt * P:(tt + 1) * P, :])
                    hnb = ptl.tile([P, C], BF16, tag="hnn", bufs=2, name=f"hnb{tt}")
                    nc.vector.tensor_scalar(hnb, xt, lnc(L_RSTD, tt), lnc(L_NEGB, tt), AL.mult, AL.add)
                    ptr = pp.tile([P, CT, P], BF16, tag="pt", name=f"ptr{tt}")
                    for ci in range(CT):
                        nc.tensor.transpose(ptr[:, ci, :], hnb[:, ci * P:(ci + 1) * P], ident)
                    nc.vector.tensor_copy(hnc[:, :, 1 + tl * P:1 + (tl + 1) * P], ptr)

            def stage_MIX(j):
                """time-mixes: xr/xk on DVE stt; xv decomposed on Pool"""
                hnc = hn_t[j]
                xk = pl.tile([P, CT, 512], BF16, tag="xk", bufs=1, name=f"xk{j}")
                xv = pl.tile([P, CT, 512], BF16, tag="xv", bufs=1, name=f"xv{j}")
                xr = pl.tile([P, CT, 512], BF16, tag="xr", bufs=1, name=f"xr{j}")
                for ci in range(CT):
                    d = ptl.tile([P, 512], BF16, tag="mixd", bufs=2, name=f"d{j}_{ci}")
                    nc.gpsimd.tensor_sub(d, hnc[:, ci, 1:513], hnc[:, ci, 0:512])
                    nc.vector.scalar_tensor_tensor(xr[:, ci, :], d, cvc(S_MR, ci), hnc[:, ci, 0:512], AL.mult, AL.add)
                    nc.vector.scalar_tensor_tensor(xk[:, ci, :], d, cvc(S_MK, ci), hnc[:, ci, 0:512], AL.mult, AL.add)
                    nc.gpsimd.tensor_mul(xv[:, ci, :], d, _bcast_free(cvc(S_MV, ci), 512))
                    nc.gpsimd.tensor_add(xv[:, ci, :], xv[:, ci, :], hnc[:, ci, 0:512])
                return xk, xv, xr

            def stage_B1a(j, xr):
                """r matmuls + sigmoid batch"""
                sra = pl.tile([P, CT, 512], BF16, tag="sr", bufs=1, name=f"sra{j}")
                for co in range(CT):
                    pr = pp.tile([P, 512], F32, tag="p2", name=f"pr{j}_{co}")
                    for ci in range(CT):
                        nc.tensor.matmul(pr, wr_sb[:, ci, co * P:(co + 1) * P], xr[:, ci, :],
                                         start=(ci == 0), stop=(ci == CT - 1))
                    if j == 0:
                        nc.vector.tensor_scalar_add(pr[:, 0:1], pr[:, 0:1], cvc(S_FR, co))
                    nc.scalar.activation(sra[:, co, :], pr, ACT.Sigmoid, bias=cvc(S_RB, co))
                return sra

            def stage_B1c(j, xk, xv, sra, fillers=()):
                """k/v matmuls + Exp batch + WKV recurrence; fillers are
                PE work (deferred short-matmul groups) interleaved per co
                to pace k/v production to the DVE consumption rate."""
                rw = ptl.tile([P, CT, 512], BF16, tag="rw", bufs=1, name=f"rw{j}")
                for co in range(CT):
                    if co < len(fillers):
                        fillers[co]()
                    pk = pp.tile([P, 512], F32, tag="p2", name=f"pk{j}_{co}")
                    pv = pp.tile([P, 512], F32, tag="p1", name=f"pv{j}_{co}")
                    for ci in range(CT):
                        nc.tensor.matmul(pk, wk_sb[:, ci, co * P:(co + 1) * P], xk[:, ci, :],
                                         start=(ci == 0), stop=(ci == CT - 1))
                    for ci in range(CT):
                        nc.tensor.matmul(pv, wv_sb[:, ci, co * P:(co + 1) * P], xv[:, ci, :],
                                         start=(ci == 0), stop=(ci == CT - 1))
                    if j == 0:
                        nc.vector.tensor_scalar_add(pk[:, 0:1], pk[:, 0:1], cvc(S_FK, co))
                        nc.vector.tensor_scalar_add(pv[:, 0:1], pv[:, 0:1], cvc(S_FV, co))
                    ek = pl.tile([P, 512], F32, tag="ek", bufs=2, name=f"ek{j}_{co}")
                    nc.scalar.activation(ek, pk, ACT.Exp, bias=cvc(S_KB, co))
                    ekv = pl.tile([P, 512], F32, tag="ekv", bufs=1, name=f"ekv{j}_{co}")
                    nc.vector.scalar_tensor_tensor(ekv, pv, cvc(S_VB, co), ek, AL.add, AL.mult)
                    a_ = pl.tile([P, 513], F32, tag="a", bufs=1, name=f"a{j}_{co}")
                    b_ = pl.tile([P, 513], F32, tag="b", bufs=1, name=f"b{j}_{co}")
                    if j == 0:
                        nc.vector.memset(a_[:, 0:1], 0.0)
                        nc.vector.memset(b_[:, 0:1], 0.0)
                    else:
                        nc.vector.tensor_copy(a_[:, 0:1], carA[:, co:co + 1])
                        nc.vector.tensor_copy(b_[:, 0:1], carB[:, co:co + 1])
                    lam_bc = _bcast_free(cvc(S_LAM, co), 512)
                    nc.vector.tensor_tensor_scan(a_[:, 1:513], lam_bc, ekv, a_[:, 0:1], AL.mult, AL.add)
                    nc.vector.tensor_tensor_scan(b_[:, 1:513], lam_bc, ek, b_[:, 0:1], AL.mult, AL.add)
                    nc.gpsimd.tensor_copy(carA[:, co:co + 1], a_[:, 512:513])
                    nc.gpsimd.tensor_copy(carB[:, co:co + 1], b_[:, 512:513])
                    # num in-place on ekv, den in-place on ek (DVE); muls on Pool
                    nc.vector.scalar_tensor_tensor(ekv, ekv, cvc(S_EU, co), a_[:, 0:512], AL.mult, AL.add)
                    nc.vector.scalar_tensor_tensor(ek, ek, cvc(S_EU, co), b_[:, 0:512], AL.mult, AL.add)
                    nc.vector.reciprocal(ek, ek)
                    nc.gpsimd.tensor_mul(ekv, ekv, ek)
                    nc.gpsimd.tensor_mul(rw[:, co, :], ekv, sra[:, co, :])
                return rw

            def alloc_o1(j):
                return ptl.tile([P, 4, C], BF16, tag="o1", bufs=1, name=f"o1_{j}")

            def make_ps_fillers(j, o1):
                """short (+rank-1) matmul groups as per-co filler closures"""
                hnc = hn_t[j]
                def mk(tl, nco):
                    def emit():
                        tt = 4 * j + tl
                        ps = pp.tile([P, 512], F32, tag="p0", name=f"ps{tt}_{nco}")
                        for ci in range(CT):
                            nc.tensor.matmul(ps, hnc[:, ci, 1 + tl * P:1 + (tl + 1) * P],
                                             sh_sb[:, ci, nco * 512:(nco + 1) * 512],
                                             start=(ci == 0), stop=False)
                        # rank-1: (mu/std)_t * srow_co ; then o1 = psum*std
                        nc.tensor.matmul(ps, musrow[0:1, tt * P:(tt + 1) * P],
                                         srow[0:1, nco * 512:(nco + 1) * 512],
                                         start=False, stop=True)
                        nc.scalar.activation(o1[:, tl, nco * 512:(nco + 1) * 512], ps,
                                             ACT.Identity, scale=lnc(L_STD, tt))
                    return emit
                return [mk(tl, nco) for tl in range(4) for nco in range(NC2)]

            def stage_A2a_wo(j, rw, o1):
                """wo matmuls + o1 assembly + spill"""
                for tl in range(4):
                    tt = 4 * j + tl
                    for nco in range(NC2):
                        pw = pp.tile([P, 512], F32, tag="p0", name=f"pw{tt}_{nco}")
                        for ci in range(CT):
                            nc.tensor.matmul(pw, rw[:, ci, tl * P:(tl + 1) * P],
                                             wo_sb[:, ci, nco * 512:(nco + 1) * 512],
                                             start=(ci == 0), stop=(ci == CT - 1))
                        o1s = o1[:, tl, nco * 512:(nco + 1) * 512]
                        nc.vector.tensor_add(o1s, o1s, pw)
                    nc.sync.dma_start(out=o1d[tt * P:(tt + 1) * P, :], in_=o1[:, tl, :])

            def stage_A2b(j, o1):
                """ln2 + gn transpose + ffn mixes + spills"""
                gnc = ptl.tile([P, CT, 513], BF16, tag="gnc", bufs=1, name=f"gnc{j}")
                if j == 0:
                    nc.vector.memset(gnc[:, :, 0:1], 0.0)
                else:
                    nc.gpsimd.tensor_copy(gnc[:, :, 0:1], gcar[:, :, :])
                for tl in range(4):
                    tt = 4 * j + tl
                    st2 = ptl.tile([P, 2, 6], F32, tag="st", bufs=2, name=f"st2_{tt}")
                    nc.vector.bn_stats(out=st2[:, 0, :], in_=o1[:, tl, 0:512])
                    nc.vector.bn_stats(out=st2[:, 1, :], in_=o1[:, tl, 512:1024])
                    mv2 = ptl.tile([P, 2], F32, tag="mv", bufs=2, name=f"mv2_{tt}")
                    nc.vector.bn_aggr(out=mv2, in_=st2)
                    lv2 = ptl.tile([P, 1], F32, tag="lv", bufs=2, name=f"lv2_{tt}")
                    nc.scalar.activation(lv2, mv2[:, 1:2], ACT.Ln, bias=epst)
                    rstd2 = ptl.tile([P, 1], F32, tag="rstd", bufs=2, name=f"rstd2_{tt}")
                    nc.scalar.activation(rstd2, lv2, ACT.Exp, scale=-0.5)
                    negb2 = ptl.tile([P, 1], F32, tag="negb", bufs=2, name=f"negb2_{tt}")
                    nc.vector.tensor_scalar(negb2, mv2[:, 0:1], rstd2, -1.0, AL.mult, AL.mult)
                    gnb = ptl.tile([P, C], BF16, tag="hnn", bufs=2, name=f"gnb{tt}")
                    nc.vector.tensor_scalar(gnb, o1[:, tl, :], rstd2, negb2, AL.mult, AL.add)
                    ptr2 = pp.tile([P, CT, P], BF16, tag="pt", name=f"ptr2_{tt}")
                    for ci in range(CT):
                        nc.tensor.transpose(ptr2[:, ci, :], gnb[:, ci * P:(ci + 1) * P], ident)
                    nc.vector.tensor_copy(gnc[:, :, 1 + tl * P:1 + (tl + 1) * P], ptr2)
                nc.gpsimd.tensor_copy(gcar[:, :, :], gnc[:, :, 512:513])
                gk = ptl.tile([P, CT, 512], BF16, tag="gk", bufs=1, name=f"gk{j}")
                gr = ptl.tile([P, CT, 512], BF16, tag="gr", bufs=1, name=f"gr{j}")
                for ci in range(CT):
                    d2 = ptl.tile([P, 512], BF16, tag="mixd", bufs=2, name=f"d2_{j}_{ci}")
                    nc.gpsimd.tensor_sub(d2, gnc[:, ci, 1:513], gnc[:, ci, 0:512])
                    nc.vector.scalar_tensor_tensor(gk[:, ci, :], d2, cvc(S_FFK, ci), gnc[:, ci, 0:512], AL.mult, AL.add)
                    nc.vector.scalar_tensor_tensor(gr[:, ci, :], d2, cvc(S_FFR, ci), gnc[:, ci, 0:512], AL.mult, AL.add)
                nc.sync.dma_start(
                    out=gkTd[:, :].rearrange("(ci p) t -> p ci t", p=P)[:, :, j * 512:(j + 1) * 512],
                    in_=gk)
                nc.sync.dma_start(
                    out=grTd[:, :].rearrange("(ci p) t -> p ci t", p=P)[:, :, j * 512:(j + 1) * 512],
                    in_=gr)

            # ---------------- main loop ----------------
            stage_A1(0)
            mix_t = {0: stage_MIX(0)}
            o1_t = {}
            wr_sb = w_sb["wr"] = ldw(pl, "wr", wrT_d)
            wk_sb = w_sb["wk"] = ldw(pl, "wk", wkT_d)
            wv_sb = w_sb["wv"] = ldw(pl, "wv", wvT_d)
            sh_sb = w_sb["sh"] = ldw(ptl, "sh", shT_d)
            wo_sb = w_sb["wo"] = ldw(ptl, "wo", woT_d)
            rw_t = {}
            for j in range(NJ):
                if j + 1 < NJ:
                    stage_A1(j + 1)
                xk, xv, xr = mix_t.pop(j)
                sra = stage_B1a(j, xr)
                if j >= 1:
                    o1p = alloc_o1(j - 1)
                    fillers = make_ps_fillers(j - 1, o1p)
                else:
                    o1p, fillers = None, ()
                rw_t[j] = stage_B1c(j, xk, xv, sra, fillers)
                if j + 1 < NJ:
                    mix_t[j + 1] = stage_MIX(j + 1)
                if j == NJ - 1:
                    # last chunk: big pool is dead after B1c; release it now so
                    # the phase-3 weight DMAs overlap the remaining tail work
                    pb_pool.release()
                    p3a = tc.alloc_tile_pool(name="p3a", bufs=1, side="right")
                    fwr_sb = ldw(p3a, "fwr", fwrT_d)
                    fwk_sb = p3a.tile([P, CT, F], BF16, tag="fwk")
                    rk = fwkT_d[:, :].rearrange("(ci p) fo -> p ci fo", p=P)
                    for q in range(4):
                        nc.sync.dma_start(out=fwk_sb[:, :, q * 1024:(q + 1) * 1024],
                                          in_=rk[:, :, q * 1024:(q + 1) * 1024])
                if j >= 1:
                    stage_A2a_wo(j - 1, rw_t.pop(j - 1), o1p)
                    stage_A2b(j - 1, o1p)

            o1_last = alloc_o1(NJ - 1)
            for f in make_ps_fillers(NJ - 1, o1_last):
                f()
            stage_A2a_wo(NJ - 1, rw_t.pop(NJ - 1), o1_last)
            stage_A2b(NJ - 1, o1_last)
            pt_pool.release()

            p3b = tc.alloc_tile_pool(name="p3b", bufs=1, side="right")
            fwv_sb = p3b.tile([P, FT, C], BF16, tag="fwv")
            rv = fwvT_d[:, :].rearrange("(fi p) co -> p fi co", p=P)
            for q in range(4):
                nc.sync.dma_start(out=fwv_sb[:, q * 8:(q + 1) * 8, :],
                                  in_=rv[:, q * 8:(q + 1) * 8, :])

            # ---------------- phase 3: FFN back ----------------
            for j in range(NJ):
                gki = p3a.tile([P, CT, 512], BF16, tag="gki", bufs=1, name=f"gki{j}")
                nc.sync.dma_start(
                    in_=gkTd[:, :].rearrange("(ci p) t -> p ci t", p=P)[:, :, j * 512:(j + 1) * 512],
                    out=gki)
                gri = p3a.tile([P, CT, 512], BF16, tag="gri", bufs=1, name=f"gri{j}")
                nc.sync.dma_start(
                    in_=grTd[:, :].rearrange("(ci p) t -> p ci t", p=P)[:, :, j * 512:(j + 1) * 512],
                    out=gri)
                # fwr matmul + sigmoid + transpose to [t,c]
                s2t = p3b.tile([P, 4, C], BF16, tag="s2t", bufs=1, name=f"s2t{j}")
                for co in range(CT):
                    prr = pp.tile([P, 512], F32, tag="p2", name=f"prr{j}_{co}")
                    for ci in range(CT):
                        nc.tensor.matmul(prr, fwr_sb[:, ci, co * P:(co + 1) * P], gri[:, ci, :],
                                         start=(ci == 0), stop=(ci == CT - 1))
                    if j == 0:
                        nc.vector.tensor_scalar_add(prr[:, 0:1], prr[:, 0:1], cvc(S_FRR, co))
                    sr2 = p3b.tile([P, 512], BF16, tag="sr2", bufs=2, name=f"sr2_{j}_{co}")
                    nc.scalar.activation(sr2, prr, ACT.Sigmoid, bias=cvc(S_RRB, co))
                    ptr3 = pp.tile([P, 4, P], BF16, tag="pt", name=f"ptr3_{j}_{co}")
                    for tl in range(4):
                        nc.tensor.transpose(ptr3[:, tl, :], sr2[:, tl * P:(tl + 1) * P], ident)
                    nc.scalar.activation(s2t[:, :, co * P:(co + 1) * P], ptr3, ACT.Copy)
                # kk2 + kv, in two t-halves of 256
                for h in range(2):
                    kk2 = p3b.tile([P, FT, 256], BF16, tag="kk2", bufs=1, name=f"kk2_{j}_{h}")
                    hs = slice(h * 256, (h + 1) * 256)
                    for ft in range(FT):
                        pkk = pp.tile([P, 256], F32, tag="p0", name=f"pkk{j}_{h}_{ft}")
                        for ci in range(CT):
                            nc.tensor.matmul(pkk, fwk_sb[:, ci, ft * P:(ft + 1) * P], gki[:, ci, hs],
                                             start=(ci == 0), stop=(ci == CT - 1))
                        if j == 0 and h == 0:
                            nc.vector.tensor_scalar_add(pkk[:, 0:1], pkk[:, 0:1], fv[:, 1, ft:ft + 1])
                        kr = p3b.tile([P, 256], BF16, tag="kr", bufs=2, name=f"kr{j}_{h}_{ft}")
                        nc.scalar.activation(kr, pkk, ACT.Relu, bias=fv[:, 0, ft:ft + 1])
                        nc.vector.tensor_mul(kk2[:, ft, :], kr, kr)
                    for tlh in range(2):
                        tl = h * 2 + tlh
                        tt = 4 * j + tl
                        o1r = p3b.tile([P, C], BF16, tag="o1r", bufs=2, name=f"o1r{tt}")
                        nc.sync.dma_start(out=o1r, in_=o1d[tt * P:(tt + 1) * P, :])
                        for nco in range(NC2):
                            pkv = pp.tile([P, 512], F32, tag="p1", name=f"pkv{tt}_{nco}")
                            for ft in range(FT):
                                nc.tensor.matmul(pkv, kk2[:, ft, tlh * P:(tlh + 1) * P],
                                                 fwv_sb[:, ft, nco * 512:(nco + 1) * 512],
                                                 start=(ft == 0), stop=(ft == FT - 1))
                            tmpv = p3b.tile([P, 512], F32, tag="kvt", bufs=2, name=f"kvt{tt}_{nco}")
                            nc.vector.tensor_mul(tmpv, pkv, s2t[:, tl, nco * 512:(nco + 1) * 512])
                            nc.vector.tensor_add(tmpv, tmpv, o1r[:, nco * 512:(nco + 1) * 512])
                            nc.sync.dma_start(out=out_d[tt * P:(tt + 1) * P, nco * 512:(nco + 1) * 512],
                                              in_=tmpv)

            p3b.release()
            p3a.release()

    nc.compile()
    return nc


_NC_CACHE = {}


def get_nc(T):
    if T not in _NC_CACHE:
        _NC_CACHE[T] = build_nc(T)
    return _NC_CACHE[T]


def host_prep(inp, T):
    """Build per-core in_maps from full inputs (float64 math on host)."""
    f8 = lambda a: np.asarray(a, np.float64)
    x = np.asarray(inp["x"], np.float32)
    w1, b1 = f8(inp["ln1_w"]), f8(inp["ln1_b"])
    w2, b2 = f8(inp["ln2_w"]), f8(inp["ln2_b"])
    Wk, Wv, Wr, Wo = f8(inp["att_Wk"]), f8(inp["att_Wv"]), f8(inp["att_Wr"]), f8(inp["att_Wo"])
    Wsh = f8(inp["short_W"])
    fWk, fWr, fWv = f8(inp["ffn_Wk"]), f8(inp["ffn_Wr"]), f8(inp["ffn_Wv"])
    mk, mvx, mr = f8(inp["att_mix_k"]), f8(inp["att_mix_v"]), f8(inp["att_mix_r"])
    fk, fr = f8(inp["ffn_mix_k"]), f8(inp["ffn_mix_r"])
    decay, first = f8(inp["att_time_decay"]), f8(inp["att_time_first"])

    def pack_c(v):
        return np.asarray(v, np.float32).reshape(CT, P).T  # [128, CT]

    lam = np.exp(-np.exp(decay))
    eu = np.exp(first)
    kbias = Wk @ b1
    vbias = Wv @ b1
    rbias = Wr @ b1
    fixk = -Wk @ ((1.0 - mk) * b1)
    fixv = -Wv @ ((1.0 - mvx) * b1)
    fixr = -Wr @ ((1.0 - mr) * b1)
    kkbias = fWk @ b2
    fixkk = -fWk @ ((1.0 - fk) * b2)
    rrbias = fWr @ b2
    fixrr = -fWr @ ((1.0 - fr) * b2)
    srow = Wsh.sum(axis=1)

    cvec = np.stack([pack_c(v) for v in
                     [lam, eu, mk, mvx, mr, kbias, vbias, rbias,
                      fixk, fixv, fixr, fk, fr, rrbias, fixrr]], axis=1)  # [128, 15, 8]
    fvec = np.stack([np.asarray(v, np.float32).reshape(FT, P).T for v in [kkbias, fixkk]],
                    axis=1)  # [128, 2, 32]

    shared = {
        "wkT": np.ascontiguousarray((Wk * w1[None, :]).T.astype(BF)),
        "wvT": np.ascontiguousarray((Wv * w1[None, :]).T.astype(BF)),
        "wrT": np.ascontiguousarray((Wr * w1[None, :]).T.astype(BF)),
        "woT": np.ascontiguousarray(Wo.T.astype(BF)),
        "shT": np.ascontiguousarray(Wsh.T.astype(BF)),
        "fwkT": np.ascontiguousarray((fWk * w2[None, :]).T.astype(BF)),
        "fwrT": np.ascontiguousarray((fWr * w2[None, :]).T.astype(BF)),
        "fwvT": np.ascontiguousarray(fWv.T.astype(BF)),
        "cvec": np.ascontiguousarray(cvec.astype(np.float32)),
        "fvec": np.ascontiguousarray(fvec.astype(np.float32)),
        "srow": np.ascontiguousarray(srow.reshape(1, C).astype(BF)),
        "ident": np.ascontiguousarray(np.eye(P, dtype=np.float32).astype(BF)),
    }
    TTl = T // P
    in_maps = []
    for b in range(x.shape[0]):
        m = dict(shared)
        xb = f8(x[b, :T, :])
        mu = xb.mean(axis=1)
        var = xb.var(axis=1)
        std = np.sqrt(var + EPS)
        rstd = 1.0 / std
        negb = -mu * rstd
        lncol = np.stack([rstd, negb, std], axis=0)          # [3, T]
        lncol = lncol.reshape(3, TTl, P).transpose(2, 0, 1)  # [128, 3, TT]
        m["lncol"] = np.ascontiguousarray(lncol.astype(np.float32))
        m["musrow"] = np.ascontiguousarray((mu * rstd).reshape(1, T).astype(BF))
        m["x"] = np.ascontiguousarray(x[b, :T, :].astype(BF))
        in_maps.append(m)
    return in_maps


def kernel(**inputs):
    T = 2048
    nc = get_nc(T)
    in_maps = host_prep(inputs, T)
    res = run_bass_kernel_spmd(nc, in_maps, core_ids=list(range(len(in_maps))))
    out = np.stack([r["out"] for r in res.results], axis=0)
    return out.astype(np.float32)
